# revision 37
# baseline (speedup 1.0000x reference)
"""Trainium2 Bass kernel for nn_GATLayer (gnn_message_passing).

Math: the reference computes
    Wh = h @ W.T
    e[i, j] = (Wh @ a_l)[i] + (Wh @ a_r)[j] + b
    out = softmax(e, axis=1) @ Wh
Because e[i, :] = const_i + t where t = Wh @ a_r, every softmax row equals
softmax(t) exactly (row-constant shifts cancel).  Hence
    out[i, :] = softmax(t) @ Wh = ((p @ h) @ W.T) / sum(p),  p = exp(t - c)
for any constant c, identical for all rows i.  The kernel computes the
weighted column-sum u = p @ h and Z = sum(p) in one streaming pass over h,
then projects y = (u @ W.T) / Z and broadcasts y to its row block.

Layout: h streams in 8 super-tiles of [128, 8, 64] where partition p holds 8
CONSECUTIVE rows (row = 1024*s + 8*p + j) — both DMA sides are then fully
linear (no sub-512B segments).  The u-matmul contracts row-class j across
partitions; PSUM accumulation over all (s, j) gives the same u.

Engine split: t = h@w_t runs as elementwise mult (Pool for 6 super-tiles,
DVE for 2) + segmented reduce (DVE);  exp on ACT;  u-accumulation on PE
(h_s as stationary, p column as moving);  finale on PE/DVE/ACT.

Toolchain constraint: walrus allows ONE sync-wait per instruction
(bacc legalizes the rest into EventSemaphores).  The tiny "funnel" copies
and junk prefetch matmuls absorb DMA-queue waits so every hot instruction
carries at most one; DMA queues are routed so no queue is reused.

Each of the 8 cores runs the same program and emits one [1024, 64] row block
(all blocks are equal); the host concatenates them to the full [8192, 64].
"""

import sys
from contextlib import ExitStack

sys.path.insert(0, "/opt/trn_rl_repo")

import numpy as np

import concourse.bass as bass
import concourse.mybir as mybir
import concourse.tile as tile
from concourse import bacc, bass_utils
from concourse.tile_rust import add_dep_helper

N = 8192
D = 64
NCORES = 8
ROWS_PER_CORE = N // NCORES  # 1024
P = 128                      # SBUF partitions
NT = N // P                  # 64 row tiles of h
SUP = 8                      # row tiles per super-tile
NSUP = NT // SUP             # 8 super-tiles
DVE_MULTS = 3                # super-tiles whose t-mult runs on DVE (rest Pool)
FP32 = mybir.dt.float32
EXP_SHIFT = -8.0             # exp(t - 8): cancels in u/Z, guards overflow


def _bcast_sup(t):
    """View a [P, D] tile as [P, SUP, D] via a step-0 middle dim."""
    return bass.AP(tensor=t.tensor, offset=t.offset, ap=[t.ap[0], [0, SUP], t.ap[1]])


def build_kernel(ctx, tc, h, W, aw, out):
    nc = tc.nc
    const = ctx.enter_context(tc.tile_pool(name="const", bufs=1))
    hpool = ctx.enter_context(tc.tile_pool(name="hbuf", bufs=NSUP))
    prpool = ctx.enter_context(tc.tile_pool(name="prod", bufs=NSUP))
    spool = ctx.enter_context(tc.tile_pool(name="scratch", bufs=2))
    tppool = ctx.enter_context(tc.tile_pool(name="tp", bufs=NSUP))
    fpool = ctx.enter_context(tc.tile_pool(name="funnel", bufs=NSUP))
    ppool = ctx.enter_context(tc.tile_pool(name="psum", bufs=1, space="PSUM"))
    upool = ctx.enter_context(tc.tile_pool(name="upsum", bufs=1, space="PSUM"))

    # --- constants.  Queue routing (no queue reused => no credit waits):
    # a_r -> SWDGE q0, h s=0 -> SWDGE q1, out -> SWDGE q2;
    # Wsb -> HWDGE q0, h s=1..7 -> HWDGE q1..q7.
    Wsb = const.tile([D, D], FP32)
    nc.sync.dma_start(out=Wsb, in_=W)
    a_r = const.tile([D, 1], FP32)
    nc.gpsimd.dma_start(out=a_r, in_=aw[0:1, D : 2 * D].rearrange("a d -> d a"))
    junk_ps = upool.tile([1, 1], FP32)
    ones_row = const.tile([1, P], FP32)
    nc.vector.memset(ones_row, 1.0)
    ones_col = const.tile([P, 1], FP32)
    nc.vector.memset(ones_col, 1.0)
    bias_sh = const.tile([P, 1], FP32)
    nc.vector.memset(bias_sh, EXP_SHIFT)

    # w_t = W.T @ a_r as a row vector, then broadcast to all partitions.
    # pre_w absorbs Wsb's DMA wait so wt_mm carries only a_r's.
    wt_ps = ppool.tile([1, D], FP32)
    pre_w = nc.tensor.matmul(
        junk_ps, lhsT=Wsb[:, 0:1], rhs=Wsb[:, 0:1], start=True, stop=True
    )
    wt_mm = nc.tensor.matmul(wt_ps, lhsT=a_r, rhs=Wsb, start=True, stop=True)
    add_dep_helper(wt_mm.ins, pre_w.ins, sync=False, reason="pe sees Wsb dma first")
    wt_row = const.tile([1, D], FP32)
    nc.vector.tensor_copy(wt_row, wt_ps)
    wrepb_ps = ppool.tile([P, D], FP32)
    nc.tensor.matmul(wrepb_ps, lhsT=ones_row, rhs=wt_row, start=True, stop=True)
    w_base = const.tile([P, D], FP32)
    nc.scalar.copy(w_base, wrepb_ps)
    # Pool replicates w_t x8 from SBUF so the Pool mults' producer is Pool
    w_rep = const.tile([P, SUP, D], FP32)
    for j in range(SUP):
        nc.gpsimd.tensor_copy(w_rep[:, j, :], w_base)
    # W.T for the final projection: 4 DVE 32x32 block transposes
    WT_sb = const.tile([D, D], FP32)
    for a in range(2):
        for b in range(2):
            nc.vector.transpose(
                out=WT_sb[32 * a : 32 * a + 32, 32 * b : 32 * b + 32],
                in_=Wsb[32 * b : 32 * b + 32, 32 * a : 32 * a + 32],
            )

    # per-super-tile partial Z sums, reduced at the end
    z_parts = const.tile([P, NSUP], FP32)
    u_ps = upool.tile([D, 1], FP32)

    # h viewed as [s][p][j][d], row = 1024*s + 8*p + j: fully linear DMA
    hview = h.rearrange("(s p j) d -> s p j d", j=SUP, p=P)

    prev_mul = {}
    for s in range(NSUP):
        h_s = hpool.tile([P, SUP, D], FP32)
        # h s=0 reuses HW queue 0 after the (tiny, long-done) Wsb DMA; its
        # only wait is that queue credit, which is within the 1-wait limit.
        nc.sync.dma_start(out=h_s, in_=hview[s])
        # t-path: prod = h_s * w_t (broadcast), then segmented sum over d.
        # The funnel copy absorbs the DMA-queue wait (1-wait walrus limit).
        # Chaining funnel s after mult s-1 (same engine) stops the scheduler
        # from hoisting all funnels first, which would stall early mults on
        # late DMAs.
        mul_eng = nc.vector if s < DVE_MULTS else nc.gpsimd
        jd = fpool.tile([1, 1], FP32)
        fun = mul_eng.tensor_copy(jd, h_s[0:1, 0, 0:1])
        if mul_eng in prev_mul:
            add_dep_helper(
                fun.ins, prev_mul[mul_eng].ins, sync=False, reason="funnel order"
            )
        prod = prpool.tile([P, SUP, D], FP32)
        mul = mul_eng.tensor_mul(prod, h_s, w_rep)
        add_dep_helper(mul.ins, fun.ins, sync=False, reason="engine sees h_s dma first")
        prev_mul[mul_eng] = mul
        t_sup = tppool.tile([P, SUP], FP32)
        nc.vector.tensor_reduce(
            out=t_sup, in_=prod, axis=mybir.AxisListType.X, op=mybir.AluOpType.add
        )
        p_sup = tppool.tile([P, SUP], FP32)
        # accum_out gives this super-tile's Z partial for free on ACT
        nc.scalar.activation(
            out=p_sup,
            in_=t_sup,
            func=mybir.ActivationFunctionType.Exp,
            bias=bias_sh,
            scale=1.0,
            accum_out=z_parts[:, s : s + 1],
        )
        # PE prefetch touching h_s: absorbs the DMA wait so the first real
        # u-matmul below carries only the ACT wait.
        pre = nc.tensor.matmul(
            junk_ps, lhsT=h_s[:, 0, 0:1], rhs=h_s[:, 0, 0:1], start=True, stop=True
        )
        for j in range(SUP):
            k = s * SUP + j
            mm = nc.tensor.matmul(
                u_ps,
                lhsT=h_s[:, j, :],
                rhs=p_sup[:, j : j + 1],
                start=(k == 0),
                stop=(k == NT - 1),
            )
            if j == 0:
                add_dep_helper(mm.ins, pre.ins, sync=False, reason="pe sees h_s dma")

    # --- finale: Z, y = (u @ W.T)/Z, broadcast to the row block ---
    zcol = spool.tile([P, 1], FP32)
    nc.vector.tensor_reduce(
        out=zcol, in_=z_parts, axis=mybir.AxisListType.X, op=mybir.AluOpType.add
    )
    z_ps = ppool.tile([1, 1], FP32)
    nc.tensor.matmul(z_ps, lhsT=zcol, rhs=ones_col, start=True, stop=True)
    recip = spool.tile([1, 1], FP32)
    nc.vector.reciprocal(recip, z_ps)
    u_sb = spool.tile([D, 1], FP32)
    nc.vector.tensor_copy(u_sb, u_ps)
    y_ps = ppool.tile([1, D], FP32)
    nc.tensor.matmul(y_ps, lhsT=u_sb, rhs=WT_sb, start=True, stop=True)
    y_sb = spool.tile([1, D], FP32)
    jf = fpool.tile([1, 1], FP32, tag="jfin")
    fun2 = nc.vector.tensor_copy(jf, y_ps[0:1, 0:1])
    ts = nc.vector.tensor_scalar_mul(out=y_sb, in0=y_ps, scalar1=recip)
    add_dep_helper(ts.ins, fun2.ins, sync=False, reason="dve sees y_ps first")
    ybc_ps = ppool.tile([P, D], FP32)
    nc.tensor.matmul(ybc_ps, lhsT=ones_row, rhs=y_sb, start=True, stop=True)
    ybc_sb = spool.tile([P, D], FP32)
    nc.scalar.copy(ybc_sb, ybc_ps)
    # partition p holds output rows 8p..8p+7 (contiguous 2KB per partition);
    # the DMA re-reads the same 64-float row 8x per partition (step-0 dim) —
    # step-0 on the DMA source is HW-proven, unlike on compute engines.
    nc.sync.dma_start(
        out=out.rearrange("(p j) d -> p j d", j=SUP), in_=_bcast_sup(ybc_sb)
    )


def build_bass():
    nc = bacc.Bacc("TRN2", debug=False, target_bir_lowering=False)
    h = nc.dram_tensor("h", [N, D], FP32, kind="ExternalInput").ap()
    W = nc.dram_tensor("W", [D, D], FP32, kind="ExternalInput").ap()
    aw = nc.dram_tensor("attn_w", [1, 2 * D], FP32, kind="ExternalInput").ap()
    out = nc.dram_tensor("out", [ROWS_PER_CORE, D], FP32, kind="ExternalOutput").ap()
    with tile.TileContext(nc) as tc:
        with ExitStack() as ctx:
            build_kernel(ctx, tc, h, W, aw, out)
    nc.compile()
    return nc


_NC_CACHE = None


def _get_nc():
    global _NC_CACHE
    if _NC_CACHE is None:
        _NC_CACHE = build_bass()
    return _NC_CACHE


def kernel(**inputs) -> np.ndarray:
    h = np.ascontiguousarray(np.asarray(inputs["h"], dtype=np.float32))
    W = np.ascontiguousarray(np.asarray(inputs["W"], dtype=np.float32))
    aw = np.ascontiguousarray(np.asarray(inputs["attn_w"], dtype=np.float32))
    assert h.shape == (N, D) and W.shape == (D, D) and aw.shape == (1, 2 * D)

    nc = _get_nc()
    in_map = {"h": h, "W": W, "attn_w": aw}
    in_maps = [in_map for _ in range(NCORES)]
    res = bass_utils.run_bass_kernel_spmd(nc, in_maps, list(range(NCORES)))
    blocks = [res.results[i]["out"] for i in range(NCORES)]
    return np.concatenate(blocks, axis=0)


if __name__ == "__main__":
    nc = _get_nc()
    print("Bass program built OK")


# revision 40
# speedup vs baseline: 1.0189x; 1.0189x over previous
"""Trainium2 Bass kernel for nn_GATLayer (gnn_message_passing).

Math: the reference computes
    Wh = h @ W.T
    e[i, j] = (Wh @ a_l)[i] + (Wh @ a_r)[j] + b
    out = softmax(e, axis=1) @ Wh
Because e[i, :] = const_i + t where t = Wh @ a_r, every softmax row equals
softmax(t) exactly (row-constant shifts cancel).  Hence
    out[i, :] = softmax(t) @ Wh = ((p @ h) @ W.T) / sum(p),  p = exp(t - c)
for any constant c, identical for all rows i.  The kernel computes the
weighted column-sum u = p @ h and Z = sum(p) in one streaming pass over h,
then projects y = (u @ W.T) / Z and broadcasts y to its row block.

Layout: h streams in 8 super-tiles of [128, 8, 64] where partition p holds 8
CONSECUTIVE rows (row = 1024*s + 8*p + j) — both DMA sides are then fully
linear (no sub-512B segments).  The u-matmul contracts row-class j across
partitions; PSUM accumulation over all (s, j) gives the same u.

Engine split: t = h@w_t runs as elementwise mult (Pool for 6 super-tiles,
DVE for 2) + segmented reduce (DVE);  exp on ACT;  u-accumulation on PE
(h_s as stationary, p column as moving);  finale on PE/DVE/ACT.

Toolchain constraint: walrus allows ONE sync-wait per instruction
(bacc legalizes the rest into EventSemaphores).  The tiny "funnel" copies
and junk prefetch matmuls absorb DMA-queue waits so every hot instruction
carries at most one; DMA queues are routed so no queue is reused.

Each of the 8 cores runs the same program and emits one [1024, 64] row block
(all blocks are equal); the host concatenates them to the full [8192, 64].
"""

import sys
from contextlib import ExitStack

sys.path.insert(0, "/opt/trn_rl_repo")

import numpy as np

import concourse.bass as bass
import concourse.mybir as mybir
import concourse.tile as tile
from concourse import bacc, bass_utils
from concourse.tile_rust import add_dep_helper

N = 8192
D = 64
NCORES = 8
ROWS_PER_CORE = N // NCORES  # 1024
P = 128                      # SBUF partitions
NT = N // P                  # 64 row tiles of h
SUP = 8                      # row tiles per super-tile
NSUP = NT // SUP             # 8 super-tiles
DVE_MULTS = 3                # super-tiles whose t-mult runs on DVE (rest Pool)
FP32 = mybir.dt.float32
EXP_SHIFT = -8.0             # exp(t - 8): cancels in u/Z, guards overflow


def _bcast_sup(t):
    """View a [P, D] tile as [P, SUP, D] via a step-0 middle dim."""
    return bass.AP(tensor=t.tensor, offset=t.offset, ap=[t.ap[0], [0, SUP], t.ap[1]])


def build_kernel(ctx, tc, h, W, aw, out):
    nc = tc.nc
    const = ctx.enter_context(tc.tile_pool(name="const", bufs=1))
    hpool = ctx.enter_context(tc.tile_pool(name="hbuf", bufs=NSUP))
    prpool = ctx.enter_context(tc.tile_pool(name="prod", bufs=NSUP))
    spool = ctx.enter_context(tc.tile_pool(name="scratch", bufs=2))
    tppool = ctx.enter_context(tc.tile_pool(name="tp", bufs=NSUP))
    fpool = ctx.enter_context(tc.tile_pool(name="funnel", bufs=NSUP))
    ppool = ctx.enter_context(tc.tile_pool(name="psum", bufs=1, space="PSUM"))
    upool = ctx.enter_context(tc.tile_pool(name="upsum", bufs=1, space="PSUM"))

    # --- constants.  Queue routing (no queue reused => no credit waits):
    # a_r -> SWDGE q0, h s=0 -> SWDGE q1, out -> SWDGE q2;
    # Wsb -> HWDGE q0, h s=1..7 -> HWDGE q1..q7.
    Wsb = const.tile([D, D], FP32)
    nc.sync.dma_start(out=Wsb, in_=W)
    a_r = const.tile([D, 1], FP32)
    nc.gpsimd.dma_start(out=a_r, in_=aw[0:1, D : 2 * D].rearrange("a d -> d a"))
    junk_ps = upool.tile([1, 1], FP32)
    ones_row = const.tile([1, P], FP32)
    nc.vector.memset(ones_row, 1.0)
    ones_col = const.tile([P, 1], FP32)
    nc.vector.memset(ones_col, 1.0)
    bias_sh = const.tile([P, 1], FP32)
    nc.vector.memset(bias_sh, EXP_SHIFT)

    # w_t = W.T @ a_r as a row vector, then broadcast to all partitions.
    # pre_w absorbs Wsb's DMA wait so wt_mm carries only a_r's.
    wt_ps = ppool.tile([1, D], FP32)
    pre_w = nc.tensor.matmul(
        junk_ps, lhsT=Wsb[:, 0:1], rhs=Wsb[:, 0:1], start=True, stop=True
    )
    wt_mm = nc.tensor.matmul(wt_ps, lhsT=a_r, rhs=Wsb, start=True, stop=True)
    add_dep_helper(wt_mm.ins, pre_w.ins, sync=False, reason="pe sees Wsb dma first")
    wt_row = const.tile([1, D], FP32)
    nc.vector.tensor_copy(wt_row, wt_ps)
    wrepb_ps = ppool.tile([P, D], FP32)
    nc.tensor.matmul(wrepb_ps, lhsT=ones_row, rhs=wt_row, start=True, stop=True)
    w_base = const.tile([P, D], FP32)
    nc.scalar.copy(w_base, wrepb_ps)
    # W.T for the final projection: 4 DVE 32x32 block transposes
    WT_sb = const.tile([D, D], FP32)
    for a in range(2):
        for b in range(2):
            nc.vector.transpose(
                out=WT_sb[32 * a : 32 * a + 32, 32 * b : 32 * b + 32],
                in_=Wsb[32 * b : 32 * b + 32, 32 * a : 32 * a + 32],
            )

    # per-super-tile partial Z sums, reduced at the end
    z_parts = const.tile([P, NSUP], FP32)
    u_ps = upool.tile([D, 1], FP32)

    # h viewed as [s][p][j][d], row = 1024*s + 8*p + j: fully linear DMA
    hview = h.rearrange("(s p j) d -> s p j d", j=SUP, p=P)

    prev_mul = {}
    for s in range(NSUP):
        h_s = hpool.tile([P, SUP, D], FP32)
        # h s=0 reuses HW queue 0 after the (tiny, long-done) Wsb DMA; its
        # only wait is that queue credit, which is within the 1-wait limit.
        nc.sync.dma_start(out=h_s, in_=hview[s])
        # t-path: prod = h_s * w_t (broadcast), then segmented sum over d.
        # The funnel copy absorbs the DMA-queue wait (1-wait walrus limit).
        # Chaining funnel s after mult s-1 (same engine) stops the scheduler
        # from hoisting all funnels first, which would stall early mults on
        # late DMAs.
        mul_eng = nc.vector if s < DVE_MULTS else nc.gpsimd
        jd = fpool.tile([1, 1], FP32)
        fun = mul_eng.tensor_copy(jd, h_s[0:1, 0, 0:1])
        if mul_eng in prev_mul:
            add_dep_helper(
                fun.ins, prev_mul[mul_eng].ins, sync=False, reason="funnel order"
            )
        prod = prpool.tile([P, SUP, D], FP32)
        mul = mul_eng.tensor_mul(prod, h_s, _bcast_sup(w_base))
        add_dep_helper(mul.ins, fun.ins, sync=False, reason="engine sees h_s dma first")
        prev_mul[mul_eng] = mul
        t_sup = tppool.tile([P, SUP], FP32)
        nc.vector.tensor_reduce(
            out=t_sup, in_=prod, axis=mybir.AxisListType.X, op=mybir.AluOpType.add
        )
        p_sup = tppool.tile([P, SUP], FP32)
        # accum_out gives this super-tile's Z partial for free on ACT
        nc.scalar.activation(
            out=p_sup,
            in_=t_sup,
            func=mybir.ActivationFunctionType.Exp,
            bias=bias_sh,
            scale=1.0,
            accum_out=z_parts[:, s : s + 1],
        )
        # PE prefetch touching h_s: absorbs the DMA wait so the first real
        # u-matmul below carries only the ACT wait.
        pre = nc.tensor.matmul(
            junk_ps, lhsT=h_s[:, 0, 0:1], rhs=h_s[:, 0, 0:1], start=True, stop=True
        )
        for j in range(SUP):
            k = s * SUP + j
            mm = nc.tensor.matmul(
                u_ps,
                lhsT=h_s[:, j, :],
                rhs=p_sup[:, j : j + 1],
                start=(k == 0),
                stop=(k == NT - 1),
            )
            if j == 0:
                add_dep_helper(mm.ins, pre.ins, sync=False, reason="pe sees h_s dma")

    # --- finale: Z, y = (u @ W.T)/Z, broadcast to the row block ---
    zcol = spool.tile([P, 1], FP32)
    nc.vector.tensor_reduce(
        out=zcol, in_=z_parts, axis=mybir.AxisListType.X, op=mybir.AluOpType.add
    )
    z_ps = ppool.tile([1, 1], FP32)
    nc.tensor.matmul(z_ps, lhsT=zcol, rhs=ones_col, start=True, stop=True)
    recip = spool.tile([1, 1], FP32)
    nc.vector.reciprocal(recip, z_ps)
    u_sb = spool.tile([D, 1], FP32)
    nc.vector.tensor_copy(u_sb, u_ps)
    y_ps = ppool.tile([1, D], FP32)
    nc.tensor.matmul(y_ps, lhsT=u_sb, rhs=WT_sb, start=True, stop=True)
    y_sb = spool.tile([1, D], FP32)
    jf = fpool.tile([1, 1], FP32, tag="jfin")
    fun2 = nc.vector.tensor_copy(jf, y_ps[0:1, 0:1])
    ts = nc.vector.tensor_scalar_mul(out=y_sb, in0=y_ps, scalar1=recip)
    add_dep_helper(ts.ins, fun2.ins, sync=False, reason="dve sees y_ps first")
    ybc_ps = ppool.tile([P, D], FP32)
    nc.tensor.matmul(ybc_ps, lhsT=ones_row, rhs=y_sb, start=True, stop=True)
    # materialize the 8 row-copies per partition with one step-0-read DVE
    # copy so the out-DMA moves fully contiguous 2KB segments
    out_sb = spool.tile([P, SUP, D], FP32)
    nc.vector.tensor_copy(out_sb, _bcast_sup(ybc_ps))
    # partition p holds output rows 8p..8p+7 (contiguous 2KB per partition)
    nc.sync.dma_start(out=out.rearrange("(p j) d -> p j d", j=SUP), in_=out_sb)


def build_bass():
    nc = bacc.Bacc("TRN2", debug=False, target_bir_lowering=False)
    h = nc.dram_tensor("h", [N, D], FP32, kind="ExternalInput").ap()
    W = nc.dram_tensor("W", [D, D], FP32, kind="ExternalInput").ap()
    aw = nc.dram_tensor("attn_w", [1, 2 * D], FP32, kind="ExternalInput").ap()
    out = nc.dram_tensor("out", [ROWS_PER_CORE, D], FP32, kind="ExternalOutput").ap()
    with tile.TileContext(nc) as tc:
        with ExitStack() as ctx:
            build_kernel(ctx, tc, h, W, aw, out)
    nc.compile()
    return nc


_NC_CACHE = None


def _get_nc():
    global _NC_CACHE
    if _NC_CACHE is None:
        _NC_CACHE = build_bass()
    return _NC_CACHE


def kernel(**inputs) -> np.ndarray:
    h = np.ascontiguousarray(np.asarray(inputs["h"], dtype=np.float32))
    W = np.ascontiguousarray(np.asarray(inputs["W"], dtype=np.float32))
    aw = np.ascontiguousarray(np.asarray(inputs["attn_w"], dtype=np.float32))
    assert h.shape == (N, D) and W.shape == (D, D) and aw.shape == (1, 2 * D)

    nc = _get_nc()
    in_map = {"h": h, "W": W, "attn_w": aw}
    in_maps = [in_map for _ in range(NCORES)]
    res = bass_utils.run_bass_kernel_spmd(nc, in_maps, list(range(NCORES)))
    blocks = [res.results[i]["out"] for i in range(NCORES)]
    return np.concatenate(blocks, axis=0)


if __name__ == "__main__":
    nc = _get_nc()
    print("Bass program built OK")
